# revision 83
# baseline (speedup 1.0000x reference)
"""Trainium2 Bass kernel for nn_Attention_40492951666725.

Full attention layer: qkv proj -> RoPE (interleaved pairs, rot dim 32) ->
softmax(QK^T)V -> out proj.  B=4, N=2048, DIM=1024, H=16, DH=64.

Sharding: 8 cores, core c handles batch b=c//2 and query-half c%2 (1024
query tokens, all 16 heads, full 2048-token K/V).  K/V projection is
computed redundantly by the two cores sharing a batch; no collectives.
The host rotates the token axis per core so the core's own query tokens
are always columns [0:1024] of xT (attention is permutation-invariant
over keys, so k/v/cos/sin just follow the same order).

Layouts (per core):
  xT   [DIM, 2048]  (host-transposed)   -> lhsT/rhs for projections
  q^T  [feat, 1024], k^T [feat, 2048]   feat on partitions
  S^T  [kj, qi]  (kj on partitions)     -> softmax via exp (no max-sub;
        scores are O(+-10) so fp32 exp is safe), P^T [kj, qi] bf16.
  AV runs output-transposed for full PE utilization: the P^T block
        [128 kj, 128 qi] is the stationary operand and V_aug [128 kj, 65]
        (64 v-feats + ones-column) streams as the moving operand, so every
        cycle writes 128 psum partitions (vs 65 in the V-stationary form).
        Accumulate over the 16 kj tiles into psum [128 qi, 4, 65]; col 64
        is the softmax denominator, applied per-partition via reciprocal +
        tensor_scalar_mul (no gpsimd broadcast needed).
  attn [qi, feat] blocks are then PE-transposed (identity matmul) into
        attnT [feat, qi] for the out projection, sharing the psA psum
        slots; out proj produces out [tok, DIM] directly.

RoPE: rotate_every_two(q) is a fixed feat-space linear map -> done with a
single [128,128] block-diagonal matmul (Rm), then q_rot = q*cos + (Rq)*sin
elementwise on DVE; pass-dims use cos=1/sin=0 so all 64 dims are uniform.
"""

import os
import numpy as np
import ml_dtypes

import concourse.bass as bass
from concourse import bacc
import concourse.tile as tile
from concourse import mybir, library_config
from concourse.bass_utils import run_bass_kernel_spmd

BF = ml_dtypes.bfloat16
F8 = ml_dtypes.float8_e4m3
bf16 = mybir.dt.bfloat16
f8e4 = mybir.dt.float8e4
f32 = mybir.dt.float32

B, N, DIM, H, DH, ROT = 4, 2048, 1024, 16, 64, 32
INNER = H * DH
NQ = N // 2            # query tokens per core
NCORES = 8
P = 128
KD = DIM // (2 * P)    # 4 DoubleRow contraction groups over model dim
KI = INNER // P        # 8 contraction tiles over inner dim (out proj, bf16)
NKT = N // P           # 16 kj partition tiles
NQT = NQ // P          # 8 qi partition tiles
HPB = H // 2           # 8 head-pair blocks

DR = mybir.MatmulPerfMode.DoubleRow
Exp = mybir.ActivationFunctionType.Exp

# power-of-2 weight prescales keep the fp8 hi/lo pair out of e4m3's
# subnormal range; all are unwound exactly (exp scale, ones-row value)
SQ, SK, SV = 256.0, 32.0, 32.0
EXP_SCALE = 1.0 / (SQ * SK)

_CACHE = {}


def _build_rope_consts(sin, cos):
    """cos_pad/sin_pad [128, N] for one head-pair feat block, Rm [128,128].

    Uses the provided sin/cos tables [N, ROT]; pass-dims get cos=1/sin=0 so
    RoPE applies uniformly over all 64 head dims."""
    cos_pad = np.ones((P, N), np.float32)
    sin_pad = np.zeros((P, N), np.float32)
    for half in range(2):                                # two heads per block
        r0 = half * DH
        cos_pad[r0:r0 + ROT, :] = cos.T
        sin_pad[r0:r0 + ROT, :] = sin.T

    # Rm[dp, d]: out[d] = sum_dp Rm[dp, d] * q[dp]  == rotate_every_two(q)[d]
    Rm = np.zeros((P, P), np.float32)
    for half in range(2):
        r0 = half * DH
        for i in range(0, ROT, 2):
            Rm[r0 + i + 1, r0 + i] = -1.0                # out[2i]   = -q[2i+1]
            Rm[r0 + i, r0 + i + 1] = 1.0                 # out[2i+1] =  q[2i]
    return cos_pad, sin_pad, Rm


def _build_program():
    nc = bacc.Bacc(trn_type="TRN2")

    xkv_d = nc.dram_tensor("xkv", [4 * P, 4, N], f8e4, kind="ExternalInput")
    wq_d = nc.dram_tensor("wq", [4 * P, 6, INNER], f8e4, kind="ExternalInput")
    wk_d = nc.dram_tensor("wk", [4 * P, 6, INNER], f8e4, kind="ExternalInput")
    wv_d = nc.dram_tensor("wv", [4 * P, 6, INNER], f8e4, kind="ExternalInput")
    wo_d = nc.dram_tensor("wo", [INNER, DIM], bf16, kind="ExternalInput")
    cosk_d = nc.dram_tensor("cosk", [P, N], bf16, kind="ExternalInput")
    sink_d = nc.dram_tensor("sink", [P, N], bf16, kind="ExternalInput")
    rm_d = nc.dram_tensor("rm", [P, P], bf16, kind="ExternalInput")
    ident_d = nc.dram_tensor("ident", [P, P], bf16, kind="ExternalInput")
    out_d = nc.dram_tensor("out", [NQ, DIM], f32, kind="ExternalOutput")
    # bf16 is fine for the small k=5..7 partial; the host adds it in f32
    out2_d = nc.dram_tensor("out2", [NQ, DIM], bf16, kind="ExternalOutput")

    with tile.TileContext(nc) as tc:
        with (
            tc.tile_pool(name="res", bufs=1) as res,          # kernel-lifetime tiles
            tc.tile_pool(name="kstream", bufs=2) as kstream,  # per-hp q/k tiles
            tc.tile_pool(name="wstream", bufs=1) as wstream,
            tc.tile_pool(name="pt", bufs=4) as ptp,           # P^T tiles
            # rope temps: t1/t2 writes and the consuming add execute in DVE
            # queue order, so a single buffer adds no stalls
            tc.tile_pool(name="tmp", bufs=1) as tmp,
            tc.tile_pool(name="stg", bufs=2) as stg,          # attn [qi, feat] stage
            tc.tile_pool(name="small", bufs=2) as small,
            tc.tile_pool(name="ostage", bufs=2) as ostage,
            tc.tile_pool(name="psA", bufs=2, space="PSUM") as psA,    # [128,512] proj/outproj/transpose
            tc.tile_pool(name="psS", bufs=2, space="PSUM") as psS,    # [128,1024] scores
            tc.tile_pool(name="psV", bufs=2, space="PSUM") as psV,    # [128,4,65] AV^T
        ):
            nc.gpsimd.load_library(library_config.attn)

            # ---- resident loads (small rope consts first, K-weights last) ----
            cosk = res.tile([P, N], bf16, tag="cosk")
            sink = res.tile([P, N], bf16, tag="sink")
            rm = res.tile([P, P], bf16, tag="rm")
            ident = res.tile([P, P], bf16, tag="ident")
            for t, d in ((rm, rm_d), (ident, ident_d)):
                nc.scalar.dma_start(t[:], d[:])
            # xkv on the SP queue, weights on the Act queue (idle at warmup).
            # Loads are split so the exact inputs of the first projection
            # tiles (query-token halves of xkv, head-block-0 weight columns)
            # land in the first few us instead of behind ~6MB of bulk DMA.
            xkv, wq, wk = [], [], []
            for k in range(KD):
                xkv.append(res.tile([P, 4, N], f8e4, tag=f"xkv{k}",
                                    name=f"xkv{k}"))
                wq.append(res.tile([P, 6, INNER], f8e4, tag=f"wq{k}",
                                   name=f"wq{k}"))
                wk.append(res.tile([P, 6, INNER], f8e4, tag=f"wk{k}",
                                   name=f"wk{k}"))
            for k in range(KD):
                nc.sync.dma_start(xkv[k][:, :, 0:512],
                                  xkv_d[k * P:(k + 1) * P, :, 0:512])
            for w, d in ((wq, wq_d), (wk, wk_d)):
                for k in range(KD):
                    nc.scalar.dma_start(w[k][:, :, 0:P],
                                        d[k * P:(k + 1) * P, :, 0:P])
            for k in range(KD):
                nc.sync.dma_start(xkv[k][:, :, 512:NQ],
                                  xkv_d[k * P:(k + 1) * P, :, 512:NQ])
            for t, d in ((cosk, cosk_d), (sink, sink_d)):
                nc.sync.dma_start(t[:], d[:])
            # xkv key-half chunks are loaded after vproj's wv weights (see
            # below): the first AV demands vaug tiles ~2us before K-proj
            # demands these columns
            for w, d in ((wq, wq_d), (wk, wk_d)):
                for k in range(KD):
                    nc.scalar.dma_start(w[k][:, :, P:INNER],
                                        d[k * P:(k + 1) * P, :, P:INNER])

            def hl_matmuls(ps, wt, wsl, xt, xsl, g):
                """Three DoubleRow matmuls accumulating one 256-row hi/lo
                group g: (wh_a,wh_a)x(xh_a,xl_a) + (wh_b,wh_b)x(xh_b,xl_b)
                + (wl_a,wl_b)x(xh_a,xh_b)."""
                nc.tensor.matmul(ps, wt[:, 0:2, wsl], xt[:, 0:2, xsl],
                                 perf_mode=DR, start=(g == 0), stop=False)
                nc.tensor.matmul(ps, wt[:, 2:4, wsl], xt[:, 2:4, xsl],
                                 perf_mode=DR, start=False, stop=False)
                nc.tensor.matmul(ps, wt[:, 4:6, wsl], xt[:, 0:4:2, xsl],
                                 perf_mode=DR, start=False, stop=(g == KD - 1))

            zz = res.tile([P, P], bf16, tag="zz")
            nc.vector.memset(zz[:], 0.0)
            attnT = []
            for k in range(KI):
                attnT.append(res.tile([P, NQ], bf16, tag=f"attnT{k}", name=f"attnT{k}"))
            vaug = []
            for mt in range(NKT):
                vt = res.tile([P, HPB, 2, 65], bf16, tag=f"vaug{mt}", name=f"vaug{mt}")
                # ones-row carries the V prescale so the normalize divide
                # unwinds it exactly: attn = sum(P*SV*v) / (SV*sum(P))
                nc.vector.memset(vt[:, :, :, 64], SV)
                vaug.append(vt)

            state = {}

            def proj_gen(hp):
                """Generator: project+rope feat block hp in small PE quanta.

                Yields between ~200-900ns chunks of PE work so the caller can
                interleave it into the exp-paced attention stream (the PE
                executes its queue in order; without interleaving, AV matmuls
                gated on Act-engine exps head-block independent proj work)."""
                c0 = hp * P
                qrot = kstream.tile([P, NQ], bf16, tag="qrot")
                krot = kstream.tile([P, N], bf16, tag="krot")
                state[hp] = (qrot, krot)
                # --- q^T block: [128 feats, NQ]  (q tokens = xkv cols 0:NQ) ---
                qraw = kstream.tile([P, NQ], bf16, tag="qraw", bufs=1)
                for n in range(NQ // 512):
                    ps = psA.tile([P, 512], f32, tag="ps")
                    for g in range(KD):
                        hl_matmuls(ps[:], wq[g], slice(c0, c0 + P),
                                   xkv[g], slice(n * 512, (n + 1) * 512), g)
                        if g % 2 == 1:
                            yield
                    nc.vector.tensor_copy(qraw[:, n * 512:(n + 1) * 512], ps[:])
                    yield
                for n in range(NQ // 512):
                    sl = slice(n * 512, (n + 1) * 512)
                    psw = psA.tile([P, 512], f32, tag="ps")
                    nc.tensor.matmul(psw[:], rm[:], qraw[:, sl], start=True, stop=True)
                    t1 = tmp.tile([P, 512], bf16, tag="t1")
                    nc.vector.tensor_mul(t1[:], qraw[:, sl], cosk[:, sl])
                    t2 = tmp.tile([P, 512], bf16, tag="t2")
                    nc.vector.tensor_mul(t2[:], psw[:], sink[:, sl])
                    nc.vector.tensor_add(qrot[:, sl], t1[:], t2[:])
                    yield
                # --- k^T block: [128 feats, N] ---
                kraw = kstream.tile([P, N], bf16, tag="kraw", bufs=1)
                for n in range(N // 512):
                    ps = psA.tile([P, 512], f32, tag="ps")
                    for g in range(KD):
                        hl_matmuls(ps[:], wk[g], slice(c0, c0 + P),
                                   xkv[g], slice(n * 512, (n + 1) * 512), g)
                        if g % 2 == 1:
                            yield
                    nc.vector.tensor_copy(kraw[:, n * 512:(n + 1) * 512], ps[:])
                    yield
                for n in range(N // 512):
                    sl = slice(n * 512, (n + 1) * 512)
                    psw = psA.tile([P, 512], f32, tag="ps")
                    nc.tensor.matmul(psw[:], rm[:], kraw[:, sl], start=True, stop=True)
                    t1 = tmp.tile([P, 512], bf16, tag="t1")
                    nc.vector.tensor_mul(t1[:], kraw[:, sl], cosk[:, sl])
                    t2 = tmp.tile([P, 512], bf16, tag="t2")
                    nc.vector.tensor_mul(t2[:], psw[:], sink[:, sl])
                    nc.vector.tensor_add(krot[:, sl], t1[:], t2[:])
                    yield

            def vproj_start(bn):
                """Issue the wv weight DMAs for vproj(bn) eagerly."""
                wvt = []
                # bn=0 loads at warmup on the SP queue behind xkv; bn=1 loads
                # mid-attention where Act paces the exps, so SP there too
                for k in range(KD):
                    t = wstream.tile([P, 6, 512], f8e4, tag=f"wv{k}", name=f"wv{k}")
                    nc.sync.dma_start(t[:], wv_d[k * P:(k + 1) * P, :,
                                               bn * 512:(bn + 1) * 512])
                    wvt.append(t)
                return wvt

            vprog = {0: 0, 1: 0}   # vaug tiles fully emitted per bn

            def vproj_gen(bn, wvt):
                """Generator: project V feats for hp blocks 4bn..4bn+3."""
                for mt in range(NKT):
                    ps = psA.tile([P, 512], f32, tag="ps")
                    msl = slice(mt * P, (mt + 1) * P)
                    for g in range(KD):
                        nc.tensor.matmul(ps[:], xkv[g][:, 0:2, msl],
                                         wvt[g][:, 0:2, :], perf_mode=DR,
                                         start=(g == 0), stop=False)
                        nc.tensor.matmul(ps[:], xkv[g][:, 2:4, msl],
                                         wvt[g][:, 2:4, :], perf_mode=DR,
                                         start=False, stop=False)
                        nc.tensor.matmul(ps[:], xkv[g][:, 0:4:2, msl],
                                         wvt[g][:, 4:6, :], perf_mode=DR,
                                         start=False, stop=(g == KD - 1))
                        if g % 2 == 1:
                            yield
                    nc.vector.tensor_copy(
                        vaug[mt][:, bn * 4:(bn + 1) * 4, :, 0:64],
                        ps[:].rearrange("p (b h d) -> p b h d", b=4, h=2))
                    vprog[bn] = mt + 1
                    yield

            stages = {}

            def emit_attention(hp, half, filler, prev_finish):
                """QK -> exp -> AV for one head-pair half, pulling filler
                quanta so the PE stream never head-blocks on Act-paced exps.
                AV for tile kt is emitted one step behind exp(kt); the
                previous phase's normalize/transpose work is emitted into
                this phase's early steps so its psum-drain and DVE/Pool
                latency hide under our QK/exp ramp."""
                qrot, krot = state[hp]
                hoff = half * DH

                def pull_one():
                    for f in list(filler):
                        try:
                            next(f)
                            return True
                        except StopIteration:
                            filler.remove(f)
                    return False

                def pull(k=1):
                    for _ in range(k):
                        if not pull_one():
                            return

                pvs = []
                pts = {}

                def emit_av(kt):
                    # program-order requirement: vaug[kt]'s write must be
                    # emitted before this read (tile deps follow trace order)
                    bn = hp // 4
                    while vprog[bn] <= kt:
                        if not pull_one():
                            break
                    for qh in range(2):
                        for qi in range(4):
                            qt = qh * 4 + qi
                            nc.tensor.matmul(pvs[qh][:, qi, :],
                                             pts[kt][:, qt * P:(qt + 1) * P],
                                             vaug[kt][:, hp, half, :],
                                             start=False,
                                             stop=(kt == NKT - 1 and qi == 3))
                    pts.pop(kt)

                fin1, fin2 = prev_finish if prev_finish else (None, None)
                for kt in range(NKT):
                    if kt == 1:
                        for qh in range(2):
                            pv = psV.tile([P, 4, 65], f32, tag="av", name="av")
                            # open one accumulation group per psum bank:
                            # group tracking is 2KB-region granular, so the 4
                            # qi sub-chains share a single start/stop pair
                            nc.tensor.matmul(pv[:, :, :], zz[:], cosk[:, 0:260],
                                             start=True, stop=False)
                            pvs.append(pv)
                    if kt > 1:
                        # hp 0 pulls harder: vproj(0) rides as filler and AV
                        # consumes one vaug tile per exp-paced step.  Later
                        # phases only need ~0.5 quanta per step; pulling more
                        # front-loads the filler and starves the last phases.
                        if hp == 0:
                            pull(2)
                        elif hp >= 6 or kt % 2 == 0:
                            pull(1)
                    ps = psS.tile([P, NQ], f32, tag="s")
                    for qn in range(NQ // 512):
                        nc.tensor.matmul(
                            ps[:, qn * 512:(qn + 1) * 512],
                            krot[hoff:hoff + DH, kt * P:(kt + 1) * P],
                            qrot[hoff:hoff + DH, qn * 512:(qn + 1) * 512],
                            start=True, stop=True)
                    pt = ptp.tile([P, NQ], bf16, tag="pt")
                    nc.scalar.activation(pt[:], ps[:], Exp, scale=EXP_SCALE)
                    pts[kt] = pt
                    if kt == 0 and fin1:
                        fin1()
                    if kt == 4 and fin2:
                        fin2()
                    if kt > 0:
                        emit_av(kt - 1)
                emit_av(NKT - 1)

                def finish1():
                    # psum -> sbuf staging (DVE), then per-row divide by the
                    # ones-column denominator on the idle gpsimd engine
                    for qh in range(2):
                        cp = small.tile([P, 4, 65], f32, tag=f"cp{qh}",
                                        name="cp", bufs=1)
                        nc.vector.tensor_copy(cp[:], pvs[qh][:])
                        for qi in range(4):
                            qt = qh * 4 + qi
                            if half == 0:
                                stages[(hp, qt)] = stg.tile(
                                    [P, P], bf16, tag=f"stg{qt}", name="stgt")
                            nc.gpsimd.normalize_recip(
                                stages[(hp, qt)][:, hoff:hoff + DH],
                                cp[:, qi, 0:64], cp[:, qi, 64:65])

                def finish2():
                    if half == 1:
                        for qt in range(NQT):
                            tr = psA.tile([P, P], bf16, tag="ps", name="tr")
                            nc.tensor.transpose(tr[:], stages.pop((hp, qt))[:],
                                                ident[:])
                            nc.vector.tensor_copy(
                                attnT[hp][:, qt * P:(qt + 1) * P], tr[:])

                return finish1, finish2

            def prefetch_wo(n):
                wot = []
                for k in range(KI):
                    t = wstream.tile([P, 512], bf16, tag=f"wo{n}_{k}",
                                     name=f"wo{n}_{k}", bufs=1)
                    nc.sync.dma_start(t[:], wo_d[k * P:(k + 1) * P,
                                                 n * 512:(n + 1) * 512])
                    wot.append(t)
                return wot

            wo_pre = {}

            def outproj1_gen():
                """Out-projection partial over inner tiles k=0..4, emitted as
                filler into the last head-pairs' exp-paced phases (attnT[0..4]
                are final once hp=4's transposes have been emitted).  The
                k=5..7 remainder goes to out2 after the last phase; the host
                adds the two partials (exact in f32)."""
                for n in range(DIM // 512):
                    wot = wo_pre[n]
                    for mt in range(NQ // P):
                        ps = psA.tile([P, 512], f32, tag="ps")
                        for k in range(6):
                            nc.tensor.matmul(ps[:],
                                             attnT[k][:, mt * P:(mt + 1) * P],
                                             wot[k][:],
                                             start=(k == 0), stop=(k == 5))
                            if k % 3 == 2:
                                yield
                        st = ostage.tile([P, 512], f32, tag="ost")
                        nc.vector.tensor_copy(st[:], ps[:])
                        nc.sync.dma_start(
                            out_d[mt * P:(mt + 1) * P, n * 512:(n + 1) * 512],
                            st[:])
                        yield

            wvt0 = vproj_start(0)
            for k in range(KD):
                nc.sync.dma_start(xkv[k][:, :, NQ:N],
                                  xkv_d[k * P:(k + 1) * P, :, NQ:N])
            for _ in proj_gen(0):
                pass
            # vproj(0) is pulled as filler by the first attention phases; AV
            # for tile kt just waits on vaug[kt]'s copy via tile deps
            filler = [vproj_gen(0, wvt0)]
            finish = None
            for hp in range(HPB):
                pg = None
                if hp + 1 < HPB:
                    pg = proj_gen(hp + 1)
                    filler.append(pg)
                if hp == 3:
                    # hp=4's AV demand-drains this; spreading it into hp>=4's
                    # phases fills their otherwise proj-less deficit
                    filler.append(vproj_gen(1, vproj_start(1)))
                if hp == 4:
                    wo_pre[0] = prefetch_wo(0)
                    wo_pre[1] = prefetch_wo(1)
                finish = emit_attention(hp, 0, filler, finish)
                if hp == 6:
                    # appended between hp=6's phases: attnT[5]'s transposes
                    # (hp=5's finisher) are emitted during (6,0), so the
                    # k<=5 reads below stay behind them in program order
                    filler.append(outproj1_gen())
                finish = emit_attention(hp, 1, filler, finish)
                state.pop(hp)
                # proj(hp+1) must be fully emitted before its attention phase
                if pg is not None:
                    for _ in pg:
                        pass
                    if pg in filler:
                        filler.remove(pg)

            # last phase's normalize + transposes, then any out-proj part-1
            # leftovers the phase pulls didn't cover
            finish[0]()
            finish[1]()
            for g in list(filler):
                for _ in g:
                    pass

            # ---- out projection remainder: out2 = attnT[5..7].T @ Wout ----
            # psums come from the now-idle psS pool, two tiles per [128,1024]
            # slot (separate 2KB banks, so separate accumulation groups);
            # with psA's 2 slots that makes enough in-flight psums that the
            # 3-matmul groups never wait on the copy+DMA drain
            ps2, st2 = None, None
            for n in range(DIM // 512):
                wot = wo_pre[n]
                for mt in range(NQ // P):
                    if mt % 2 == 0:
                        ps2 = psS.tile([P, NQ], f32, tag="s")
                        st2 = ostage.tile([P, 2, 512], bf16, tag="ost2")
                    ps = ps2[:, (mt % 2) * 512:(mt % 2) * 512 + 512]
                    for k in range(6, KI):
                        nc.tensor.matmul(ps, attnT[k][:, mt * P:(mt + 1) * P],
                                         wot[k][:],
                                         start=(k == 6), stop=(k == KI - 1))
                    # stage copies alternate between DVE and the now-idle Act
                    # engine; pair-merged bf16 DMAs halve the per-call queue
                    # overhead (seq+DGE ~1.2us each) that paced the old tail
                    if mt % 2 == 0:
                        nc.vector.tensor_copy(st2[:, 0, :], ps)
                    else:
                        nc.scalar.copy(st2[:, 1, :], ps)
                        eng = nc.sync if mt % 4 == 1 else nc.scalar
                        eng.dma_start(
                            out2_d.rearrange("(a p) m -> p a m", p=P)[
                                :, mt - 1:mt + 1,
                                n * 512:(n + 1) * 512], st2[:])

    nc.compile()
    return nc


def _hilo(m):
    """fp8 hi + lo residual pair of [1024, C] -> H, L [4, 2, 128, C] f32
    grouped as (group g, chunk a/b, partition, col)."""
    h8 = m.astype(F8)
    l8 = (m - h8.astype(np.float32)).astype(F8)
    return (h8.reshape(4, 2, P, -1), l8.reshape(4, 2, P, -1))


def _pack_x(m):
    """[1024, N] -> [512, 4, N] fp8, slots (xh_a, xl_a, xh_b, xl_b)."""
    H, L = _hilo(m)
    out = np.stack([H[:, 0], L[:, 0], H[:, 1], L[:, 1]], axis=2)
    return np.ascontiguousarray(out.reshape(4 * P, 4, -1))


def _pack_w(m):
    """[1024, C] -> [512, 6, C] fp8, slots (h_a, h_a, h_b, h_b, l_a, l_b)."""
    H, L = _hilo(m)
    out = np.stack([H[:, 0], H[:, 0], H[:, 1], H[:, 1], L[:, 0], L[:, 1]],
                   axis=2)
    return np.ascontiguousarray(out.reshape(4 * P, 6, -1))


def _prep_inputs(x, sin, cos, Wqkv, Wout):
    """Host-side sharding/layout prep. Returns in_maps list for 8 cores."""
    x = np.asarray(x, np.float32)
    Wqkv = np.asarray(Wqkv, np.float32)
    Wout = np.asarray(Wout, np.float32)
    scale = DH ** -0.5
    wq = _pack_w(Wqkv[:, :INNER] * (scale * SQ))
    wk = _pack_w(Wqkv[:, INNER:2 * INNER] * SK)
    wv = _pack_w(Wqkv[:, 2 * INNER:] * SV)
    wo = Wout.astype(BF)
    cos_pad, sin_pad, Rm = _build_rope_consts(
        np.asarray(sin, np.float32), np.asarray(cos, np.float32))
    rm = Rm.astype(BF)
    ident = np.eye(P, dtype=BF)

    in_maps = []
    for c in range(NCORES):
        b, half = divmod(c, 2)
        xT = np.ascontiguousarray(x[b].T)                          # [DIM, N]
        ck, sk = cos_pad, sin_pad
        if half == 1:        # rotate tokens so this core's queries come first
            xT = np.concatenate([xT[:, NQ:], xT[:, :NQ]], axis=1)
            ck = np.concatenate([ck[:, NQ:], ck[:, :NQ]], axis=1)
            sk = np.concatenate([sk[:, NQ:], sk[:, :NQ]], axis=1)
        in_maps.append({
            "xkv": _pack_x(xT),
            "wq": wq, "wk": wk, "wv": wv, "wo": wo,
            "cosk": np.ascontiguousarray(ck).astype(BF),
            "sink": np.ascontiguousarray(sk).astype(BF),
            "rm": rm, "ident": ident,
        })
    return in_maps


LAST_RESULTS = None


def kernel(x, sin, cos, Wqkv, Wout):
    global LAST_RESULTS
    if "nc" not in _CACHE:
        _CACHE["nc"] = _build_program()
    nc = _CACHE["nc"]
    in_maps = _prep_inputs(x, sin, cos, Wqkv, Wout)
    trace = bool(int(os.environ.get("KERNEL_TRACE", "0")))
    try:
        res = run_bass_kernel_spmd(nc, in_maps, core_ids=list(range(NCORES)),
                                   trace=trace)
    except (ImportError, ModuleNotFoundError):
        # NTFF profiling hook unavailable in this environment
        res = run_bass_kernel_spmd(nc, in_maps, core_ids=list(range(NCORES)),
                                   trace=False)
    LAST_RESULTS = res
    out = np.empty((B, N, DIM), np.float32)
    for c in range(NCORES):
        b, half = divmod(c, 2)
        out[b, half * NQ:(half + 1) * NQ, :] = (
            res.results[c]["out"] + res.results[c]["out2"])
    return out


# revision 84
# speedup vs baseline: 1.0028x; 1.0028x over previous
"""Trainium2 Bass kernel for nn_Attention_40492951666725.

Full attention layer: qkv proj -> RoPE (interleaved pairs, rot dim 32) ->
softmax(QK^T)V -> out proj.  B=4, N=2048, DIM=1024, H=16, DH=64.

Sharding: 8 cores, core c handles batch b=c//2 and query-half c%2 (1024
query tokens, all 16 heads, full 2048-token K/V).  K/V projection is
computed redundantly by the two cores sharing a batch; no collectives.
The host rotates the token axis per core so the core's own query tokens
are always columns [0:1024] of xT (attention is permutation-invariant
over keys, so k/v/cos/sin just follow the same order).

Layouts (per core):
  xT   [DIM, 2048]  (host-transposed)   -> lhsT/rhs for projections
  q^T  [feat, 1024], k^T [feat, 2048]   feat on partitions
  S^T  [kj, qi]  (kj on partitions)     -> softmax via exp (no max-sub;
        scores are O(+-10) so fp32 exp is safe), P^T [kj, qi] bf16.
  AV runs output-transposed for full PE utilization: the P^T block
        [128 kj, 128 qi] is the stationary operand and V_aug [128 kj, 65]
        (64 v-feats + ones-column) streams as the moving operand, so every
        cycle writes 128 psum partitions (vs 65 in the V-stationary form).
        Accumulate over the 16 kj tiles into psum [128 qi, 4, 65]; col 64
        is the softmax denominator, applied per-partition via reciprocal +
        tensor_scalar_mul (no gpsimd broadcast needed).
  attn [qi, feat] blocks are then PE-transposed (identity matmul) into
        attnT [feat, qi] for the out projection, sharing the psA psum
        slots; out proj produces out [tok, DIM] directly.

RoPE: rotate_every_two(q) is a fixed feat-space linear map -> done with a
single [128,128] block-diagonal matmul (Rm), then q_rot = q*cos + (Rq)*sin
elementwise on DVE; pass-dims use cos=1/sin=0 so all 64 dims are uniform.
"""

import os
import numpy as np
import ml_dtypes

import concourse.bass as bass
from concourse import bacc
import concourse.tile as tile
from concourse import mybir, library_config
from concourse.bass_utils import run_bass_kernel_spmd

BF = ml_dtypes.bfloat16
F8 = ml_dtypes.float8_e4m3
bf16 = mybir.dt.bfloat16
f8e4 = mybir.dt.float8e4
f32 = mybir.dt.float32

B, N, DIM, H, DH, ROT = 4, 2048, 1024, 16, 64, 32
INNER = H * DH
NQ = N // 2            # query tokens per core
NCORES = 8
P = 128
KD = DIM // (2 * P)    # 4 DoubleRow contraction groups over model dim
KI = INNER // P        # 8 contraction tiles over inner dim (out proj, bf16)
NKT = N // P           # 16 kj partition tiles
NQT = NQ // P          # 8 qi partition tiles
HPB = H // 2           # 8 head-pair blocks

DR = mybir.MatmulPerfMode.DoubleRow
Exp = mybir.ActivationFunctionType.Exp

# power-of-2 weight prescales keep the fp8 hi/lo pair out of e4m3's
# subnormal range; all are unwound exactly (exp scale, ones-row value)
SQ, SK, SV = 256.0, 32.0, 32.0
EXP_SCALE = 1.0 / (SQ * SK)

_CACHE = {}


def _build_rope_consts(sin, cos):
    """cos_pad/sin_pad [128, N] for one head-pair feat block, Rm [128,128].

    Uses the provided sin/cos tables [N, ROT]; pass-dims get cos=1/sin=0 so
    RoPE applies uniformly over all 64 head dims."""
    cos_pad = np.ones((P, N), np.float32)
    sin_pad = np.zeros((P, N), np.float32)
    for half in range(2):                                # two heads per block
        r0 = half * DH
        cos_pad[r0:r0 + ROT, :] = cos.T
        sin_pad[r0:r0 + ROT, :] = sin.T

    # Rm[dp, d]: out[d] = sum_dp Rm[dp, d] * q[dp]  == rotate_every_two(q)[d]
    Rm = np.zeros((P, P), np.float32)
    for half in range(2):
        r0 = half * DH
        for i in range(0, ROT, 2):
            Rm[r0 + i + 1, r0 + i] = -1.0                # out[2i]   = -q[2i+1]
            Rm[r0 + i, r0 + i + 1] = 1.0                 # out[2i+1] =  q[2i]
    return cos_pad, sin_pad, Rm


def _build_program():
    nc = bacc.Bacc(trn_type="TRN2")

    xkv_d = nc.dram_tensor("xkv", [4 * P, 4, N], f8e4, kind="ExternalInput")
    wq_d = nc.dram_tensor("wq", [4 * P, 6, INNER], f8e4, kind="ExternalInput")
    wk_d = nc.dram_tensor("wk", [4 * P, 6, INNER], f8e4, kind="ExternalInput")
    wv_d = nc.dram_tensor("wv", [4 * P, 6, INNER], f8e4, kind="ExternalInput")
    wo_d = nc.dram_tensor("wo", [INNER, DIM], bf16, kind="ExternalInput")
    cosk_d = nc.dram_tensor("cosk", [P, N], bf16, kind="ExternalInput")
    sink_d = nc.dram_tensor("sink", [P, N], bf16, kind="ExternalInput")
    rm_d = nc.dram_tensor("rm", [P, P], bf16, kind="ExternalInput")
    ident_d = nc.dram_tensor("ident", [P, P], bf16, kind="ExternalInput")
    out_d = nc.dram_tensor("out", [NQ, DIM], f32, kind="ExternalOutput")
    # bf16 is fine for the small k=5..7 partial; the host adds it in f32
    out2_d = nc.dram_tensor("out2", [NQ, DIM], bf16, kind="ExternalOutput")

    with tile.TileContext(nc) as tc:
        with (
            tc.tile_pool(name="res", bufs=1) as res,          # kernel-lifetime tiles
            tc.tile_pool(name="kstream", bufs=2) as kstream,  # per-hp q/k tiles
            tc.tile_pool(name="wstream", bufs=1) as wstream,
            tc.tile_pool(name="pt", bufs=4) as ptp,           # P^T tiles
            # rope temps: t1/t2 writes and the consuming add execute in DVE
            # queue order, so a single buffer adds no stalls
            tc.tile_pool(name="tmp", bufs=1) as tmp,
            tc.tile_pool(name="stg", bufs=2) as stg,          # attn [qi, feat] stage
            tc.tile_pool(name="small", bufs=2) as small,
            tc.tile_pool(name="ostage", bufs=2) as ostage,
            tc.tile_pool(name="psA", bufs=2, space="PSUM") as psA,    # [128,512] proj/outproj/transpose
            tc.tile_pool(name="psS", bufs=2, space="PSUM") as psS,    # [128,1024] scores
            tc.tile_pool(name="psV", bufs=2, space="PSUM") as psV,    # [128,4,65] AV^T
        ):
            nc.gpsimd.load_library(library_config.attn)

            # ---- resident loads (small rope consts first, K-weights last) ----
            cosk = res.tile([P, N], bf16, tag="cosk")
            sink = res.tile([P, N], bf16, tag="sink")
            rm = res.tile([P, P], bf16, tag="rm")
            ident = res.tile([P, P], bf16, tag="ident")
            for t, d in ((rm, rm_d), (ident, ident_d)):
                nc.scalar.dma_start(t[:], d[:])
            # xkv on the SP queue, weights on the Act queue (idle at warmup).
            # Loads are split so the exact inputs of the first projection
            # tiles (query-token halves of xkv, head-block-0 weight columns)
            # land in the first few us instead of behind ~6MB of bulk DMA.
            xkv, wq, wk = [], [], []
            for k in range(KD):
                xkv.append(res.tile([P, 4, N], f8e4, tag=f"xkv{k}",
                                    name=f"xkv{k}"))
                wq.append(res.tile([P, 6, INNER], f8e4, tag=f"wq{k}",
                                   name=f"wq{k}"))
                wk.append(res.tile([P, 6, INNER], f8e4, tag=f"wk{k}",
                                   name=f"wk{k}"))
            for k in range(KD):
                nc.sync.dma_start(xkv[k][:, :, 0:512],
                                  xkv_d[k * P:(k + 1) * P, :, 0:512])
            for w, d in ((wq, wq_d), (wk, wk_d)):
                for k in range(KD):
                    nc.scalar.dma_start(w[k][:, :, 0:P],
                                        d[k * P:(k + 1) * P, :, 0:P])
            for k in range(KD):
                nc.sync.dma_start(xkv[k][:, :, 512:NQ],
                                  xkv_d[k * P:(k + 1) * P, :, 512:NQ])
            for t, d in ((cosk, cosk_d), (sink, sink_d)):
                nc.sync.dma_start(t[:], d[:])
            for w, d in ((wq, wq_d), (wk, wk_d)):
                for k in range(KD):
                    nc.scalar.dma_start(w[k][:, :, P:INNER],
                                        d[k * P:(k + 1) * P, :, P:INNER])

            def hl_matmuls(ps, wt, wsl, xt, xsl, g):
                """Three DoubleRow matmuls accumulating one 256-row hi/lo
                group g: (wh_a,wh_a)x(xh_a,xl_a) + (wh_b,wh_b)x(xh_b,xl_b)
                + (wl_a,wl_b)x(xh_a,xh_b)."""
                nc.tensor.matmul(ps, wt[:, 0:2, wsl], xt[:, 0:2, xsl],
                                 perf_mode=DR, start=(g == 0), stop=False)
                nc.tensor.matmul(ps, wt[:, 2:4, wsl], xt[:, 2:4, xsl],
                                 perf_mode=DR, start=False, stop=False)
                nc.tensor.matmul(ps, wt[:, 4:6, wsl], xt[:, 0:4:2, xsl],
                                 perf_mode=DR, start=False, stop=(g == KD - 1))

            zz = res.tile([P, P], bf16, tag="zz")
            nc.vector.memset(zz[:], 0.0)
            attnT = []
            for k in range(KI):
                attnT.append(res.tile([P, NQ], bf16, tag=f"attnT{k}", name=f"attnT{k}"))
            vaug = []
            for mt in range(NKT):
                vt = res.tile([P, HPB, 2, 65], bf16, tag=f"vaug{mt}", name=f"vaug{mt}")
                # ones-row carries the V prescale so the normalize divide
                # unwinds it exactly: attn = sum(P*SV*v) / (SV*sum(P))
                nc.vector.memset(vt[:, :, :, 64], SV)
                vaug.append(vt)

            state = {}

            def proj_gen(hp):
                """Generator: project+rope feat block hp in small PE quanta.

                Yields between ~200-900ns chunks of PE work so the caller can
                interleave it into the exp-paced attention stream (the PE
                executes its queue in order; without interleaving, AV matmuls
                gated on Act-engine exps head-block independent proj work)."""
                c0 = hp * P
                qrot = kstream.tile([P, NQ], bf16, tag="qrot")
                krot = kstream.tile([P, N], bf16, tag="krot")
                state[hp] = (qrot, krot)
                # --- q^T block: [128 feats, NQ]  (q tokens = xkv cols 0:NQ) ---
                qraw = kstream.tile([P, NQ], bf16, tag="qraw", bufs=1)
                for n in range(NQ // 512):
                    ps = psA.tile([P, 512], f32, tag="ps")
                    for g in range(KD):
                        hl_matmuls(ps[:], wq[g], slice(c0, c0 + P),
                                   xkv[g], slice(n * 512, (n + 1) * 512), g)
                        if g % 2 == 1:
                            yield
                    nc.vector.tensor_copy(qraw[:, n * 512:(n + 1) * 512], ps[:])
                    yield
                for n in range(NQ // 512):
                    sl = slice(n * 512, (n + 1) * 512)
                    psw = psA.tile([P, 512], f32, tag="ps")
                    nc.tensor.matmul(psw[:], rm[:], qraw[:, sl], start=True, stop=True)
                    t1 = tmp.tile([P, 512], bf16, tag="t1")
                    nc.vector.tensor_mul(t1[:], qraw[:, sl], cosk[:, sl])
                    t2 = tmp.tile([P, 512], bf16, tag="t2")
                    nc.vector.tensor_mul(t2[:], psw[:], sink[:, sl])
                    nc.vector.tensor_add(qrot[:, sl], t1[:], t2[:])
                    yield
                # --- k^T block: [128 feats, N] ---
                kraw = kstream.tile([P, N], bf16, tag="kraw", bufs=1)
                for n in range(N // 512):
                    ps = psA.tile([P, 512], f32, tag="ps")
                    for g in range(KD):
                        hl_matmuls(ps[:], wk[g], slice(c0, c0 + P),
                                   xkv[g], slice(n * 512, (n + 1) * 512), g)
                        if g % 2 == 1:
                            yield
                    nc.vector.tensor_copy(kraw[:, n * 512:(n + 1) * 512], ps[:])
                    yield
                for n in range(N // 512):
                    sl = slice(n * 512, (n + 1) * 512)
                    psw = psA.tile([P, 512], f32, tag="ps")
                    nc.tensor.matmul(psw[:], rm[:], kraw[:, sl], start=True, stop=True)
                    t1 = tmp.tile([P, 512], bf16, tag="t1")
                    nc.vector.tensor_mul(t1[:], kraw[:, sl], cosk[:, sl])
                    t2 = tmp.tile([P, 512], bf16, tag="t2")
                    nc.vector.tensor_mul(t2[:], psw[:], sink[:, sl])
                    nc.vector.tensor_add(krot[:, sl], t1[:], t2[:])
                    yield

            def vproj_start(bn):
                """Issue the wv weight DMAs for vproj(bn) eagerly."""
                wvt = []
                # bn=0 loads at warmup on the SP queue behind xkv; bn=1 loads
                # mid-attention where Act paces the exps, so SP there too
                for k in range(KD):
                    t = wstream.tile([P, 6, 512], f8e4, tag=f"wv{k}", name=f"wv{k}")
                    nc.sync.dma_start(t[:], wv_d[k * P:(k + 1) * P, :,
                                               bn * 512:(bn + 1) * 512])
                    wvt.append(t)
                return wvt

            vprog = {0: 0, 1: 0}   # vaug tiles fully emitted per bn

            def vproj_gen(bn, wvt):
                """Generator: project V feats for hp blocks 4bn..4bn+3."""
                for mt in range(NKT):
                    ps = psA.tile([P, 512], f32, tag="ps")
                    msl = slice(mt * P, (mt + 1) * P)
                    for g in range(KD):
                        nc.tensor.matmul(ps[:], xkv[g][:, 0:2, msl],
                                         wvt[g][:, 0:2, :], perf_mode=DR,
                                         start=(g == 0), stop=False)
                        nc.tensor.matmul(ps[:], xkv[g][:, 2:4, msl],
                                         wvt[g][:, 2:4, :], perf_mode=DR,
                                         start=False, stop=False)
                        nc.tensor.matmul(ps[:], xkv[g][:, 0:4:2, msl],
                                         wvt[g][:, 4:6, :], perf_mode=DR,
                                         start=False, stop=(g == KD - 1))
                        if g % 2 == 1:
                            yield
                    nc.vector.tensor_copy(
                        vaug[mt][:, bn * 4:(bn + 1) * 4, :, 0:64],
                        ps[:].rearrange("p (b h d) -> p b h d", b=4, h=2))
                    vprog[bn] = mt + 1
                    yield

            stages = {}

            def emit_attention(hp, half, filler, prev_finish):
                """QK -> exp -> AV for one head-pair half, pulling filler
                quanta so the PE stream never head-blocks on Act-paced exps.
                AV for tile kt is emitted one step behind exp(kt); the
                previous phase's normalize/transpose work is emitted into
                this phase's early steps so its psum-drain and DVE/Pool
                latency hide under our QK/exp ramp."""
                qrot, krot = state[hp]
                hoff = half * DH

                def pull_one():
                    for f in list(filler):
                        try:
                            next(f)
                            return True
                        except StopIteration:
                            filler.remove(f)
                    return False

                def pull(k=1):
                    for _ in range(k):
                        if not pull_one():
                            return

                pvs = []
                pts = {}

                def emit_av(kt):
                    # program-order requirement: vaug[kt]'s write must be
                    # emitted before this read (tile deps follow trace order)
                    bn = hp // 4
                    while vprog[bn] <= kt:
                        if not pull_one():
                            break
                    for qh in range(2):
                        for qi in range(4):
                            qt = qh * 4 + qi
                            nc.tensor.matmul(pvs[qh][:, qi, :],
                                             pts[kt][:, qt * P:(qt + 1) * P],
                                             vaug[kt][:, hp, half, :],
                                             start=False,
                                             stop=(kt == NKT - 1 and qi == 3))
                    pts.pop(kt)

                fin1, fin2 = prev_finish if prev_finish else (None, None)
                for kt in range(NKT):
                    if kt == 1:
                        for qh in range(2):
                            pv = psV.tile([P, 4, 65], f32, tag="av", name="av")
                            # open one accumulation group per psum bank:
                            # group tracking is 2KB-region granular, so the 4
                            # qi sub-chains share a single start/stop pair
                            nc.tensor.matmul(pv[:, :, :], zz[:], cosk[:, 0:260],
                                             start=True, stop=False)
                            pvs.append(pv)
                    if kt > 1:
                        # hp 0 pulls harder: vproj(0) rides as filler and AV
                        # consumes one vaug tile per exp-paced step.  Later
                        # phases only need ~0.5 quanta per step; pulling more
                        # front-loads the filler and starves the last phases.
                        if hp == 0:
                            pull(2)
                        elif hp >= 6 or kt % 2 == 0:
                            pull(1)
                    ps = psS.tile([P, NQ], f32, tag="s")
                    for qn in range(NQ // 512):
                        nc.tensor.matmul(
                            ps[:, qn * 512:(qn + 1) * 512],
                            krot[hoff:hoff + DH, kt * P:(kt + 1) * P],
                            qrot[hoff:hoff + DH, qn * 512:(qn + 1) * 512],
                            start=True, stop=True)
                    pt = ptp.tile([P, NQ], bf16, tag="pt")
                    nc.scalar.activation(pt[:], ps[:], Exp, scale=EXP_SCALE)
                    pts[kt] = pt
                    if kt == 0 and fin1:
                        fin1()
                    if kt == 4 and fin2:
                        fin2()
                    if kt > 0:
                        emit_av(kt - 1)
                emit_av(NKT - 1)

                def finish1():
                    # psum -> sbuf staging (DVE), then per-row divide by the
                    # ones-column denominator on the idle gpsimd engine
                    for qh in range(2):
                        cp = small.tile([P, 4, 65], f32, tag=f"cp{qh}",
                                        name="cp", bufs=1)
                        nc.vector.tensor_copy(cp[:], pvs[qh][:])
                        for qi in range(4):
                            qt = qh * 4 + qi
                            if half == 0:
                                stages[(hp, qt)] = stg.tile(
                                    [P, P], bf16, tag=f"stg{qt}", name="stgt")
                            nc.gpsimd.normalize_recip(
                                stages[(hp, qt)][:, hoff:hoff + DH],
                                cp[:, qi, 0:64], cp[:, qi, 64:65])

                def finish2():
                    if half == 1:
                        for qt in range(NQT):
                            tr = psA.tile([P, P], bf16, tag="ps", name="tr")
                            nc.tensor.transpose(tr[:], stages.pop((hp, qt))[:],
                                                ident[:])
                            nc.vector.tensor_copy(
                                attnT[hp][:, qt * P:(qt + 1) * P], tr[:])

                return finish1, finish2

            def prefetch_wo(n):
                wot = []
                for k in range(KI):
                    t = wstream.tile([P, 512], bf16, tag=f"wo{n}_{k}",
                                     name=f"wo{n}_{k}", bufs=1)
                    nc.sync.dma_start(t[:], wo_d[k * P:(k + 1) * P,
                                                 n * 512:(n + 1) * 512])
                    wot.append(t)
                return wot

            wo_pre = {}

            def outproj1_gen():
                """Out-projection partial over inner tiles k=0..4, emitted as
                filler into the last head-pairs' exp-paced phases (attnT[0..4]
                are final once hp=4's transposes have been emitted).  The
                k=5..7 remainder goes to out2 after the last phase; the host
                adds the two partials (exact in f32)."""
                for n in range(DIM // 512):
                    wot = wo_pre[n]
                    for mt in range(NQ // P):
                        ps = psA.tile([P, 512], f32, tag="ps")
                        for k in range(5):
                            nc.tensor.matmul(ps[:],
                                             attnT[k][:, mt * P:(mt + 1) * P],
                                             wot[k][:],
                                             start=(k == 0), stop=(k == 4))
                            if k % 3 == 2:
                                yield
                        st = ostage.tile([P, 512], f32, tag="ost")
                        nc.vector.tensor_copy(st[:], ps[:])
                        nc.sync.dma_start(
                            out_d[mt * P:(mt + 1) * P, n * 512:(n + 1) * 512],
                            st[:])
                        yield

            wvt0 = vproj_start(0)
            # xkv key-half columns load after vproj's wv weights: the first
            # AV demands vaug tiles ~2us before K-proj demands these columns
            for k in range(KD):
                nc.sync.dma_start(xkv[k][:, :, NQ:N],
                                  xkv_d[k * P:(k + 1) * P, :, NQ:N])
            for _ in proj_gen(0):
                pass
            # vproj(0) is pulled as filler by the first attention phases; AV
            # for tile kt just waits on vaug[kt]'s copy via tile deps
            filler = [vproj_gen(0, wvt0)]
            finish = None
            for hp in range(HPB):
                pg = None
                if hp + 1 < HPB:
                    pg = proj_gen(hp + 1)
                    filler.append(pg)
                if hp == 3:
                    # hp=4's AV demand-drains this; spreading it into hp>=4's
                    # phases fills their otherwise proj-less deficit
                    filler.append(vproj_gen(1, vproj_start(1)))
                if hp == 4:
                    wo_pre[0] = prefetch_wo(0)
                    wo_pre[1] = prefetch_wo(1)
                if hp == 6:
                    filler.append(outproj1_gen())
                finish = emit_attention(hp, 0, filler, finish)
                finish = emit_attention(hp, 1, filler, finish)
                state.pop(hp)
                # proj(hp+1) must be fully emitted before its attention phase
                if pg is not None:
                    for _ in pg:
                        pass
                    if pg in filler:
                        filler.remove(pg)

            # last phase's normalize + transposes, then any out-proj part-1
            # leftovers the phase pulls didn't cover
            finish[0]()
            finish[1]()
            for g in list(filler):
                for _ in g:
                    pass

            # ---- out projection remainder: out2 = attnT[5..7].T @ Wout ----
            # psums come from the now-idle psS pool, two tiles per [128,1024]
            # slot (separate 2KB banks, so separate accumulation groups);
            # with psA's 2 slots that makes enough in-flight psums that the
            # 3-matmul groups never wait on the copy+DMA drain
            ps2, st2 = None, None
            for n in range(DIM // 512):
                wot = wo_pre[n]
                for mt in range(NQ // P):
                    if mt % 2 == 0:
                        ps2 = psS.tile([P, NQ], f32, tag="s")
                        st2 = ostage.tile([P, 2, 512], bf16, tag="ost2")
                    ps = ps2[:, (mt % 2) * 512:(mt % 2) * 512 + 512]
                    for k in range(5, KI):
                        nc.tensor.matmul(ps, attnT[k][:, mt * P:(mt + 1) * P],
                                         wot[k][:],
                                         start=(k == 5), stop=(k == KI - 1))
                    # stage copies alternate between DVE and the now-idle Act
                    # engine; pair-merged bf16 DMAs halve the per-call queue
                    # overhead (seq+DGE ~1.2us each) that paced the old tail
                    if mt % 2 == 0:
                        nc.vector.tensor_copy(st2[:, 0, :], ps)
                    else:
                        nc.scalar.copy(st2[:, 1, :], ps)
                        eng = nc.sync if mt % 4 == 1 else nc.scalar
                        eng.dma_start(
                            out2_d.rearrange("(a p) m -> p a m", p=P)[
                                :, mt - 1:mt + 1,
                                n * 512:(n + 1) * 512], st2[:])

    nc.compile()
    return nc


def _hilo(m):
    """fp8 hi + lo residual pair of [1024, C] -> H, L [4, 2, 128, C] f32
    grouped as (group g, chunk a/b, partition, col)."""
    h8 = m.astype(F8)
    l8 = (m - h8.astype(np.float32)).astype(F8)
    return (h8.reshape(4, 2, P, -1), l8.reshape(4, 2, P, -1))


def _pack_x(m):
    """[1024, N] -> [512, 4, N] fp8, slots (xh_a, xl_a, xh_b, xl_b)."""
    H, L = _hilo(m)
    out = np.stack([H[:, 0], L[:, 0], H[:, 1], L[:, 1]], axis=2)
    return np.ascontiguousarray(out.reshape(4 * P, 4, -1))


def _pack_w(m):
    """[1024, C] -> [512, 6, C] fp8, slots (h_a, h_a, h_b, h_b, l_a, l_b)."""
    H, L = _hilo(m)
    out = np.stack([H[:, 0], H[:, 0], H[:, 1], H[:, 1], L[:, 0], L[:, 1]],
                   axis=2)
    return np.ascontiguousarray(out.reshape(4 * P, 6, -1))


def _prep_inputs(x, sin, cos, Wqkv, Wout):
    """Host-side sharding/layout prep. Returns in_maps list for 8 cores."""
    x = np.asarray(x, np.float32)
    Wqkv = np.asarray(Wqkv, np.float32)
    Wout = np.asarray(Wout, np.float32)
    scale = DH ** -0.5
    wq = _pack_w(Wqkv[:, :INNER] * (scale * SQ))
    wk = _pack_w(Wqkv[:, INNER:2 * INNER] * SK)
    wv = _pack_w(Wqkv[:, 2 * INNER:] * SV)
    wo = Wout.astype(BF)
    cos_pad, sin_pad, Rm = _build_rope_consts(
        np.asarray(sin, np.float32), np.asarray(cos, np.float32))
    rm = Rm.astype(BF)
    ident = np.eye(P, dtype=BF)

    in_maps = []
    for c in range(NCORES):
        b, half = divmod(c, 2)
        xT = np.ascontiguousarray(x[b].T)                          # [DIM, N]
        ck, sk = cos_pad, sin_pad
        if half == 1:        # rotate tokens so this core's queries come first
            xT = np.concatenate([xT[:, NQ:], xT[:, :NQ]], axis=1)
            ck = np.concatenate([ck[:, NQ:], ck[:, :NQ]], axis=1)
            sk = np.concatenate([sk[:, NQ:], sk[:, :NQ]], axis=1)
        in_maps.append({
            "xkv": _pack_x(xT),
            "wq": wq, "wk": wk, "wv": wv, "wo": wo,
            "cosk": np.ascontiguousarray(ck).astype(BF),
            "sink": np.ascontiguousarray(sk).astype(BF),
            "rm": rm, "ident": ident,
        })
    return in_maps


LAST_RESULTS = None


def kernel(x, sin, cos, Wqkv, Wout):
    global LAST_RESULTS
    if "nc" not in _CACHE:
        _CACHE["nc"] = _build_program()
    nc = _CACHE["nc"]
    in_maps = _prep_inputs(x, sin, cos, Wqkv, Wout)
    trace = bool(int(os.environ.get("KERNEL_TRACE", "0")))
    try:
        res = run_bass_kernel_spmd(nc, in_maps, core_ids=list(range(NCORES)),
                                   trace=trace)
    except (ImportError, ModuleNotFoundError):
        # NTFF profiling hook unavailable in this environment
        res = run_bass_kernel_spmd(nc, in_maps, core_ids=list(range(NCORES)),
                                   trace=False)
    LAST_RESULTS = res
    out = np.empty((B, N, DIM), np.float32)
    for c in range(NCORES):
        b, half = divmod(c, 2)
        out[b, half * NQ:(half + 1) * NQ, :] = (
            res.results[c]["out"] + res.results[c]["out2"])
    return out


# revision 88
# speedup vs baseline: 1.0135x; 1.0107x over previous
"""Trainium2 Bass kernel for nn_Attention_40492951666725.

Full attention layer: qkv proj -> RoPE (interleaved pairs, rot dim 32) ->
softmax(QK^T)V -> out proj.  B=4, N=2048, DIM=1024, H=16, DH=64.

Sharding: 8 cores, core c handles batch b=c//2 and query-half c%2 (1024
query tokens, all 16 heads, full 2048-token K/V).  K/V projection is
computed redundantly by the two cores sharing a batch; no collectives.
The host rotates the token axis per core so the core's own query tokens
are always columns [0:1024] of xT (attention is permutation-invariant
over keys, so k/v/cos/sin just follow the same order).

Layouts (per core):
  xT   [DIM, 2048]  (host-transposed)   -> lhsT/rhs for projections
  q^T  [feat, 1024], k^T [feat, 2048]   feat on partitions
  S^T  [kj, qi]  (kj on partitions)     -> softmax via exp (no max-sub;
        scores are O(+-10) so fp32 exp is safe), P^T [kj, qi] bf16.
  AV runs output-transposed for full PE utilization: the P^T block
        [128 kj, 128 qi] is the stationary operand and V_aug [128 kj, 65]
        (64 v-feats + ones-column) streams as the moving operand, so every
        cycle writes 128 psum partitions (vs 65 in the V-stationary form).
        Accumulate over the 16 kj tiles into psum [128 qi, 4, 65]; col 64
        is the softmax denominator, applied per-partition via reciprocal +
        tensor_scalar_mul (no gpsimd broadcast needed).
  attn [qi, feat] blocks are then PE-transposed (identity matmul) into
        attnT [feat, qi] for the out projection, sharing the psA psum
        slots; out proj produces out [tok, DIM] directly.

RoPE: rotate_every_two(q) is a fixed feat-space linear map -> done with a
single [128,128] block-diagonal matmul (Rm), then q_rot = q*cos + (Rq)*sin
elementwise on DVE; pass-dims use cos=1/sin=0 so all 64 dims are uniform.
"""

import os
import numpy as np
import ml_dtypes

import concourse.bass as bass
from concourse import bacc
import concourse.tile as tile
from concourse import mybir, library_config
from concourse.bass_utils import run_bass_kernel_spmd

BF = ml_dtypes.bfloat16
F8 = ml_dtypes.float8_e4m3
bf16 = mybir.dt.bfloat16
f8e4 = mybir.dt.float8e4
f32 = mybir.dt.float32

B, N, DIM, H, DH, ROT = 4, 2048, 1024, 16, 64, 32
INNER = H * DH
NQ = N // 2            # query tokens per core
NCORES = 8
P = 128
KD = DIM // (2 * P)    # 4 DoubleRow contraction groups over model dim
KI = INNER // P        # 8 contraction tiles over inner dim (out proj, bf16)
NKT = N // P           # 16 kj partition tiles
NQT = NQ // P          # 8 qi partition tiles
HPB = H // 2           # 8 head-pair blocks

DR = mybir.MatmulPerfMode.DoubleRow
Exp = mybir.ActivationFunctionType.Exp

# power-of-2 weight prescales keep the fp8 hi/lo pair out of e4m3's
# subnormal range; all are unwound exactly (exp scale, ones-row value)
SQ, SK, SV = 256.0, 32.0, 32.0
EXP_SCALE = 1.0 / (SQ * SK)

_CACHE = {}


def _build_rope_consts(sin, cos):
    """cos_pad/sin_pad [128, N] for one head-pair feat block, Rm [128,128].

    Uses the provided sin/cos tables [N, ROT]; pass-dims get cos=1/sin=0 so
    RoPE applies uniformly over all 64 head dims."""
    cos_pad = np.ones((P, N), np.float32)
    sin_pad = np.zeros((P, N), np.float32)
    for half in range(2):                                # two heads per block
        r0 = half * DH
        cos_pad[r0:r0 + ROT, :] = cos.T
        sin_pad[r0:r0 + ROT, :] = sin.T

    # Rm[dp, d]: out[d] = sum_dp Rm[dp, d] * q[dp]  == rotate_every_two(q)[d]
    Rm = np.zeros((P, P), np.float32)
    for half in range(2):
        r0 = half * DH
        for i in range(0, ROT, 2):
            Rm[r0 + i + 1, r0 + i] = -1.0                # out[2i]   = -q[2i+1]
            Rm[r0 + i, r0 + i + 1] = 1.0                 # out[2i+1] =  q[2i]
    return cos_pad, sin_pad, Rm


def _build_program():
    nc = bacc.Bacc(trn_type="TRN2")

    xkv_d = nc.dram_tensor("xkv", [4 * P, 4, N], f8e4, kind="ExternalInput")
    wq_d = nc.dram_tensor("wq", [4 * P, 6, INNER], f8e4, kind="ExternalInput")
    wk_d = nc.dram_tensor("wk", [4 * P, 6, INNER], f8e4, kind="ExternalInput")
    wv_d = nc.dram_tensor("wv", [4 * P, 6, INNER], f8e4, kind="ExternalInput")
    wo_d = nc.dram_tensor("wo", [INNER, DIM], bf16, kind="ExternalInput")
    cosk_d = nc.dram_tensor("cosk", [P, N], bf16, kind="ExternalInput")
    sink_d = nc.dram_tensor("sink", [P, N], bf16, kind="ExternalInput")
    rm_d = nc.dram_tensor("rm", [P, P], bf16, kind="ExternalInput")
    ident_d = nc.dram_tensor("ident", [P, P], bf16, kind="ExternalInput")
    out_d = nc.dram_tensor("out", [NQ, DIM], f32, kind="ExternalOutput")
    # bf16 is fine for the small k=5..7 partial; the host adds it in f32
    out2_d = nc.dram_tensor("out2", [NQ, DIM], bf16, kind="ExternalOutput")

    with tile.TileContext(nc) as tc:
        with (
            tc.tile_pool(name="res", bufs=1) as res,          # kernel-lifetime tiles
            tc.tile_pool(name="kstream", bufs=2) as kstream,  # per-hp q/k tiles
            tc.tile_pool(name="wstream", bufs=1) as wstream,
            tc.tile_pool(name="pt", bufs=4) as ptp,           # P^T tiles
            # rope temps: t1/t2 writes and the consuming add execute in DVE
            # queue order, so a single buffer adds no stalls
            tc.tile_pool(name="tmp", bufs=1) as tmp,
            tc.tile_pool(name="stg", bufs=2) as stg,          # attn [qi, feat] stage
            tc.tile_pool(name="small", bufs=2) as small,
            tc.tile_pool(name="ostage", bufs=2) as ostage,
            tc.tile_pool(name="psA", bufs=2, space="PSUM") as psA,    # [128,512] proj/outproj/transpose
            tc.tile_pool(name="psS", bufs=2, space="PSUM") as psS,    # [128,1024] scores
            tc.tile_pool(name="psV", bufs=2, space="PSUM") as psV,    # [128,4,65] AV^T
        ):
            nc.gpsimd.load_library(library_config.attn)

            # ---- resident loads (small rope consts first, K-weights last) ----
            cosk = res.tile([P, N], bf16, tag="cosk")
            sink = res.tile([P, N], bf16, tag="sink")
            rm = res.tile([P, P], bf16, tag="rm")
            ident = res.tile([P, P], bf16, tag="ident")
            for t, d in ((rm, rm_d), (ident, ident_d)):
                nc.scalar.dma_start(t[:], d[:])
            # xkv on the SP queue, weights on the Act queue (idle at warmup).
            # Loads are split so the exact inputs of the first projection
            # tiles (query-token halves of xkv, head-block-0 weight columns)
            # land in the first few us instead of behind ~6MB of bulk DMA.
            xkv, wq, wk = [], [], []
            for k in range(KD):
                xkv.append(res.tile([P, 4, N], f8e4, tag=f"xkv{k}",
                                    name=f"xkv{k}"))
                wq.append(res.tile([P, 6, INNER], f8e4, tag=f"wq{k}",
                                   name=f"wq{k}"))
                wk.append(res.tile([P, 6, INNER], f8e4, tag=f"wk{k}",
                                   name=f"wk{k}"))
            for k in range(KD):
                nc.sync.dma_start(xkv[k][:, :, 0:512],
                                  xkv_d[k * P:(k + 1) * P, :, 0:512])
            for w, d in ((wq, wq_d), (wk, wk_d)):
                for k in range(KD):
                    nc.scalar.dma_start(w[k][:, :, 0:P],
                                        d[k * P:(k + 1) * P, :, 0:P])
            for k in range(KD):
                nc.sync.dma_start(xkv[k][:, :, 512:NQ],
                                  xkv_d[k * P:(k + 1) * P, :, 512:NQ])
            for t, d in ((cosk, cosk_d), (sink, sink_d)):
                nc.sync.dma_start(t[:], d[:])
            for k in range(KD):
                nc.sync.dma_start(xkv[k][:, :, NQ:N],
                                  xkv_d[k * P:(k + 1) * P, :, NQ:N])
            for w, d in ((wq, wq_d), (wk, wk_d)):
                for k in range(KD):
                    nc.scalar.dma_start(w[k][:, :, P:INNER],
                                        d[k * P:(k + 1) * P, :, P:INNER])

            def hl_matmuls(ps, wt, wsl, xt, xsl, g):
                """Three DoubleRow matmuls accumulating one 256-row hi/lo
                group g: (wh_a,wh_a)x(xh_a,xl_a) + (wh_b,wh_b)x(xh_b,xl_b)
                + (wl_a,wl_b)x(xh_a,xh_b)."""
                nc.tensor.matmul(ps, wt[:, 0:2, wsl], xt[:, 0:2, xsl],
                                 perf_mode=DR, start=(g == 0), stop=False)
                nc.tensor.matmul(ps, wt[:, 2:4, wsl], xt[:, 2:4, xsl],
                                 perf_mode=DR, start=False, stop=False)
                nc.tensor.matmul(ps, wt[:, 4:6, wsl], xt[:, 0:4:2, xsl],
                                 perf_mode=DR, start=False, stop=(g == KD - 1))

            zz = res.tile([P, 2, P], f8e4, tag="zz")
            nc.vector.memset(zz[:], 0.0)
            attnT = []
            for k in range(KI):
                attnT.append(res.tile([P, NQ], bf16, tag=f"attnT{k}", name=f"attnT{k}"))
            vaug = []
            for mt in range(NKT):
                vt = res.tile([P, HPB, 2, 65], bf16, tag=f"vaug{mt}", name=f"vaug{mt}")
                # ones-row carries the V prescale so the normalize divide
                # unwinds it exactly: attn = sum(P*SV*v) / (SV*sum(P))
                nc.vector.memset(vt[:, :, :, 64], SV)
                vaug.append(vt)

            state = {}

            def proj_gen(hp):
                """Generator: project+rope feat block hp in small PE quanta.

                Yields between ~200-900ns chunks of PE work so the caller can
                interleave it into the exp-paced attention stream (the PE
                executes its queue in order; without interleaving, AV matmuls
                gated on Act-engine exps head-block independent proj work)."""
                c0 = hp * P
                qrot = kstream.tile([P, NQ], bf16, tag="qrot")
                krot = kstream.tile([P, N], bf16, tag="krot")
                state[hp] = (qrot, krot)
                # --- q^T block: [128 feats, NQ]  (q tokens = xkv cols 0:NQ) ---
                qraw = kstream.tile([P, NQ], bf16, tag="qraw", bufs=1)
                for n in range(NQ // 512):
                    ps = psA.tile([P, 512], f32, tag="ps")
                    for g in range(KD):
                        hl_matmuls(ps[:], wq[g], slice(c0, c0 + P),
                                   xkv[g], slice(n * 512, (n + 1) * 512), g)
                        if g % 2 == 1:
                            yield
                    nc.vector.tensor_copy(qraw[:, n * 512:(n + 1) * 512], ps[:])
                    yield
                for n in range(NQ // 512):
                    sl = slice(n * 512, (n + 1) * 512)
                    psw = psA.tile([P, 512], f32, tag="ps")
                    nc.tensor.matmul(psw[:], rm[:], qraw[:, sl], start=True, stop=True)
                    t1 = tmp.tile([P, 512], bf16, tag="t1")
                    nc.vector.tensor_mul(t1[:], qraw[:, sl], cosk[:, sl])
                    t2 = tmp.tile([P, 512], bf16, tag="t2")
                    nc.vector.tensor_mul(t2[:], psw[:], sink[:, sl])
                    nc.vector.tensor_add(qrot[:, sl], t1[:], t2[:])
                    yield
                # --- k^T block: [128 feats, N] ---
                kraw = kstream.tile([P, N], bf16, tag="kraw", bufs=1)
                for n in range(N // 512):
                    ps = psA.tile([P, 512], f32, tag="ps")
                    for g in range(KD):
                        hl_matmuls(ps[:], wk[g], slice(c0, c0 + P),
                                   xkv[g], slice(n * 512, (n + 1) * 512), g)
                        if g % 2 == 1:
                            yield
                    nc.vector.tensor_copy(kraw[:, n * 512:(n + 1) * 512], ps[:])
                    yield
                for n in range(N // 512):
                    sl = slice(n * 512, (n + 1) * 512)
                    psw = psA.tile([P, 512], f32, tag="ps")
                    nc.tensor.matmul(psw[:], rm[:], kraw[:, sl], start=True, stop=True)
                    t1 = tmp.tile([P, 512], bf16, tag="t1")
                    nc.vector.tensor_mul(t1[:], kraw[:, sl], cosk[:, sl])
                    t2 = tmp.tile([P, 512], bf16, tag="t2")
                    nc.vector.tensor_mul(t2[:], psw[:], sink[:, sl])
                    nc.vector.tensor_add(krot[:, sl], t1[:], t2[:])
                    yield

            def vproj_start(bn):
                """Issue the wv weight DMAs for vproj(bn) eagerly."""
                wvt = []
                # bn=0 loads at warmup on the SP queue behind xkv; bn=1 loads
                # mid-attention where Act paces the exps, so SP there too
                for k in range(KD):
                    t = wstream.tile([P, 6, 512], f8e4, tag=f"wv{k}", name=f"wv{k}")
                    nc.sync.dma_start(t[:], wv_d[k * P:(k + 1) * P, :,
                                               bn * 512:(bn + 1) * 512])
                    wvt.append(t)
                return wvt

            vprog = {0: 0, 1: 0}   # vaug tiles fully emitted per bn

            def vproj_gen(bn, wvt):
                """Generator: project V feats for hp blocks 4bn..4bn+3."""
                for mt in range(NKT):
                    ps = psA.tile([P, 512], f32, tag="ps")
                    msl = slice(mt * P, (mt + 1) * P)
                    for g in range(KD):
                        nc.tensor.matmul(ps[:], xkv[g][:, 0:2, msl],
                                         wvt[g][:, 0:2, :], perf_mode=DR,
                                         start=(g == 0), stop=False)
                        nc.tensor.matmul(ps[:], xkv[g][:, 2:4, msl],
                                         wvt[g][:, 2:4, :], perf_mode=DR,
                                         start=False, stop=False)
                        nc.tensor.matmul(ps[:], xkv[g][:, 0:4:2, msl],
                                         wvt[g][:, 4:6, :], perf_mode=DR,
                                         start=False, stop=(g == KD - 1))
                        if g % 2 == 1:
                            yield
                    nc.vector.tensor_copy(
                        vaug[mt][:, bn * 4:(bn + 1) * 4, :, 0:64],
                        ps[:].rearrange("p (b h d) -> p b h d", b=4, h=2))
                    vprog[bn] = mt + 1
                    yield

            stages = {}

            def emit_attention(hp, half, filler, prev_finish):
                """QK -> exp -> AV for one head-pair half, pulling filler
                quanta so the PE stream never head-blocks on Act-paced exps.
                AV for tile kt is emitted one step behind exp(kt); the
                previous phase's normalize/transpose work is emitted into
                this phase's early steps so its psum-drain and DVE/Pool
                latency hide under our QK/exp ramp."""
                qrot, krot = state[hp]
                hoff = half * DH

                def pull_one():
                    for f in list(filler):
                        try:
                            next(f)
                            return True
                        except StopIteration:
                            filler.remove(f)
                    return False

                def pull(k=1):
                    for _ in range(k):
                        if not pull_one():
                            return

                pvs = []
                pts = {}

                def emit_av(kt):
                    # program-order requirement: vaug[kt]'s write must be
                    # emitted before this read (tile deps follow trace order)
                    bn = hp // 4
                    while vprog[bn] <= kt:
                        if not pull_one():
                            break
                    for qh in range(2):
                        for qi in range(4):
                            qt = qh * 4 + qi
                            nc.tensor.matmul(pvs[qh][:, qi, :],
                                             pts[kt][:, qt * P:(qt + 1) * P],
                                             vaug[kt][:, hp, half, :],
                                             start=False,
                                             stop=(kt == NKT - 1 and qi == 3))
                    pts.pop(kt)

                fin1, fin2 = prev_finish if prev_finish else (None, None)
                for kt in range(NKT):
                    if kt == 1:
                        for qh in range(2):
                            pv = psV.tile([P, 4, 65], f32, tag="av", name="av")
                            # open one accumulation group per psum bank:
                            # group tracking is 2KB-region granular, so the 4
                            # qi sub-chains share a single start/stop pair
                            nc.tensor.matmul(pv[:, :, :], zz[:],
                                             xkv[0][:, 0:2, 0:260],
                                             perf_mode=DR,
                                             start=True, stop=False)
                            pvs.append(pv)
                    if kt > 1:
                        # hp 0 pulls harder: vproj(0) rides as filler and AV
                        # consumes one vaug tile per exp-paced step.  Later
                        # phases only need ~0.5 quanta per step; pulling more
                        # front-loads the filler and starves the last phases.
                        if hp == 0:
                            pull(2)
                        elif hp >= 6 or kt % 2 == 0:
                            pull(1)
                    ps = psS.tile([P, NQ], f32, tag="s")
                    for qn in range(NQ // 512):
                        nc.tensor.matmul(
                            ps[:, qn * 512:(qn + 1) * 512],
                            krot[hoff:hoff + DH, kt * P:(kt + 1) * P],
                            qrot[hoff:hoff + DH, qn * 512:(qn + 1) * 512],
                            start=True, stop=True)
                    pt = ptp.tile([P, NQ], bf16, tag="pt")
                    nc.scalar.activation(pt[:], ps[:], Exp, scale=EXP_SCALE)
                    pts[kt] = pt
                    if kt == 0 and fin1:
                        fin1()
                    if kt == 4 and fin2:
                        fin2()
                    if kt > 0:
                        emit_av(kt - 1)
                emit_av(NKT - 1)

                def finish1():
                    # psum -> sbuf staging (DVE), then per-row divide by the
                    # ones-column denominator on the idle gpsimd engine
                    for qh in range(2):
                        cp = small.tile([P, 4, 65], f32, tag=f"cp{qh}",
                                        name="cp", bufs=1)
                        nc.vector.tensor_copy(cp[:], pvs[qh][:])
                        for qi in range(4):
                            qt = qh * 4 + qi
                            if half == 0:
                                stages[(hp, qt)] = stg.tile(
                                    [P, P], bf16, tag=f"stg{qt}", name="stgt")
                            nc.gpsimd.normalize_recip(
                                stages[(hp, qt)][:, hoff:hoff + DH],
                                cp[:, qi, 0:64], cp[:, qi, 64:65])

                def finish2():
                    if half == 1:
                        for qt in range(NQT):
                            tr = psA.tile([P, P], bf16, tag="ps", name="tr")
                            nc.tensor.transpose(tr[:], stages.pop((hp, qt))[:],
                                                ident[:])
                            nc.vector.tensor_copy(
                                attnT[hp][:, qt * P:(qt + 1) * P], tr[:])

                return finish1, finish2

            def prefetch_wo(n):
                wot = []
                for k in range(KI):
                    t = wstream.tile([P, 512], bf16, tag=f"wo{n}_{k}",
                                     name=f"wo{n}_{k}", bufs=1)
                    nc.sync.dma_start(t[:], wo_d[k * P:(k + 1) * P,
                                                 n * 512:(n + 1) * 512])
                    wot.append(t)
                return wot

            wo_pre = {}

            def outproj1_gen():
                """Out-projection partial over inner tiles k=0..4, emitted as
                filler into the last head-pairs' exp-paced phases (attnT[0..4]
                are final once hp=4's transposes have been emitted).  The
                k=5..7 remainder goes to out2 after the last phase; the host
                adds the two partials (exact in f32)."""
                for n in range(DIM // 512):
                    wot = wo_pre[n]
                    for mt in range(NQ // P):
                        ps = psA.tile([P, 512], f32, tag="ps")
                        for k in range(5):
                            nc.tensor.matmul(ps[:],
                                             attnT[k][:, mt * P:(mt + 1) * P],
                                             wot[k][:],
                                             start=(k == 0), stop=(k == 4))
                            if k % 3 == 2:
                                yield
                        st = ostage.tile([P, 512], f32, tag="ost")
                        nc.vector.tensor_copy(st[:], ps[:])
                        nc.sync.dma_start(
                            out_d[mt * P:(mt + 1) * P, n * 512:(n + 1) * 512],
                            st[:])
                        yield

            wvt0 = vproj_start(0)
            for _ in proj_gen(0):
                pass
            # vproj(0) is pulled as filler by the first attention phases; AV
            # for tile kt just waits on vaug[kt]'s copy via tile deps
            filler = [vproj_gen(0, wvt0)]
            finish = None
            for hp in range(HPB):
                pg = None
                if hp + 1 < HPB:
                    pg = proj_gen(hp + 1)
                    filler.append(pg)
                if hp == 3:
                    # hp=4's AV demand-drains this; spreading it into hp>=4's
                    # phases fills their otherwise proj-less deficit
                    filler.append(vproj_gen(1, vproj_start(1)))
                if hp == 4:
                    wo_pre[0] = prefetch_wo(0)
                    wo_pre[1] = prefetch_wo(1)
                if hp == 6:
                    filler.append(outproj1_gen())
                finish = emit_attention(hp, 0, filler, finish)
                finish = emit_attention(hp, 1, filler, finish)
                state.pop(hp)
                # proj(hp+1) must be fully emitted before its attention phase
                if pg is not None:
                    for _ in pg:
                        pass
                    if pg in filler:
                        filler.remove(pg)

            # last phase's normalize + transposes, then any out-proj part-1
            # leftovers the phase pulls didn't cover
            finish[0]()
            finish[1]()
            for g in list(filler):
                for _ in g:
                    pass

            # ---- out projection remainder: out2 = attnT[5..7].T @ Wout ----
            # psums come from the now-idle psS pool, two tiles per [128,1024]
            # slot (separate 2KB banks, so separate accumulation groups);
            # with psA's 2 slots that makes enough in-flight psums that the
            # 3-matmul groups never wait on the copy+DMA drain
            ps2, st2 = None, None
            for n in range(DIM // 512):
                wot = wo_pre[n]
                for mt in range(NQ // P):
                    if mt % 2 == 0:
                        ps2 = psS.tile([P, NQ], f32, tag="s")
                        st2 = ostage.tile([P, 2, 512], bf16, tag="ost2")
                    ps = ps2[:, (mt % 2) * 512:(mt % 2) * 512 + 512]
                    for k in range(5, KI):
                        nc.tensor.matmul(ps, attnT[k][:, mt * P:(mt + 1) * P],
                                         wot[k][:],
                                         start=(k == 5), stop=(k == KI - 1))
                    # stage copies alternate between DVE and the now-idle Act
                    # engine; pair-merged bf16 DMAs halve the per-call queue
                    # overhead (seq+DGE ~1.2us each) that paced the old tail
                    if mt % 2 == 0:
                        nc.vector.tensor_copy(st2[:, 0, :], ps)
                    else:
                        # both copies on DVE: putting one on Act made the Act
                        # queue (copy + every-other DMA) the ~2us/pair pacer
                        nc.vector.tensor_copy(st2[:, 1, :], ps)
                        eng = nc.sync if mt % 4 == 1 else nc.scalar
                        eng.dma_start(
                            out2_d.rearrange("(a p) m -> p a m", p=P)[
                                :, mt - 1:mt + 1,
                                n * 512:(n + 1) * 512], st2[:])

    nc.compile()
    return nc


def _hilo(m):
    """fp8 hi + lo residual pair of [1024, C] -> H, L [4, 2, 128, C] f32
    grouped as (group g, chunk a/b, partition, col)."""
    h8 = m.astype(F8)
    l8 = (m - h8.astype(np.float32)).astype(F8)
    return (h8.reshape(4, 2, P, -1), l8.reshape(4, 2, P, -1))


def _pack_x(m):
    """[1024, N] -> [512, 4, N] fp8, slots (xh_a, xl_a, xh_b, xl_b)."""
    H, L = _hilo(m)
    out = np.stack([H[:, 0], L[:, 0], H[:, 1], L[:, 1]], axis=2)
    return np.ascontiguousarray(out.reshape(4 * P, 4, -1))


def _pack_w(m):
    """[1024, C] -> [512, 6, C] fp8, slots (h_a, h_a, h_b, h_b, l_a, l_b)."""
    H, L = _hilo(m)
    out = np.stack([H[:, 0], H[:, 0], H[:, 1], H[:, 1], L[:, 0], L[:, 1]],
                   axis=2)
    return np.ascontiguousarray(out.reshape(4 * P, 6, -1))


def _prep_inputs(x, sin, cos, Wqkv, Wout):
    """Host-side sharding/layout prep. Returns in_maps list for 8 cores."""
    x = np.asarray(x, np.float32)
    Wqkv = np.asarray(Wqkv, np.float32)
    Wout = np.asarray(Wout, np.float32)
    scale = DH ** -0.5
    wq = _pack_w(Wqkv[:, :INNER] * (scale * SQ))
    wk = _pack_w(Wqkv[:, INNER:2 * INNER] * SK)
    wv = _pack_w(Wqkv[:, 2 * INNER:] * SV)
    wo = Wout.astype(BF)
    cos_pad, sin_pad, Rm = _build_rope_consts(
        np.asarray(sin, np.float32), np.asarray(cos, np.float32))
    rm = Rm.astype(BF)
    ident = np.eye(P, dtype=BF)

    in_maps = []
    for c in range(NCORES):
        b, half = divmod(c, 2)
        xT = np.ascontiguousarray(x[b].T)                          # [DIM, N]
        ck, sk = cos_pad, sin_pad
        if half == 1:        # rotate tokens so this core's queries come first
            xT = np.concatenate([xT[:, NQ:], xT[:, :NQ]], axis=1)
            ck = np.concatenate([ck[:, NQ:], ck[:, :NQ]], axis=1)
            sk = np.concatenate([sk[:, NQ:], sk[:, :NQ]], axis=1)
        in_maps.append({
            "xkv": _pack_x(xT),
            "wq": wq, "wk": wk, "wv": wv, "wo": wo,
            "cosk": np.ascontiguousarray(ck).astype(BF),
            "sink": np.ascontiguousarray(sk).astype(BF),
            "rm": rm, "ident": ident,
        })
    return in_maps


LAST_RESULTS = None


def kernel(x, sin, cos, Wqkv, Wout):
    global LAST_RESULTS
    if "nc" not in _CACHE:
        _CACHE["nc"] = _build_program()
    nc = _CACHE["nc"]
    in_maps = _prep_inputs(x, sin, cos, Wqkv, Wout)
    trace = bool(int(os.environ.get("KERNEL_TRACE", "0")))
    try:
        res = run_bass_kernel_spmd(nc, in_maps, core_ids=list(range(NCORES)),
                                   trace=trace)
    except (ImportError, ModuleNotFoundError):
        # NTFF profiling hook unavailable in this environment
        res = run_bass_kernel_spmd(nc, in_maps, core_ids=list(range(NCORES)),
                                   trace=False)
    LAST_RESULTS = res
    out = np.empty((B, N, DIM), np.float32)
    for c in range(NCORES):
        b, half = divmod(c, 2)
        out[b, half * NQ:(half + 1) * NQ, :] = (
            res.results[c]["out"] + res.results[c]["out2"])
    return out
